# revision 1
# baseline (speedup 1.0000x reference)
"""Rotary multi-head attention (b=8, n=1024, dim=768, heads=12, d_head=64)
on 8 Trainium2 NeuronCores, data-parallel over batch (1 batch row per core).

Per-core pipeline (dense matmuls in float32r ~tf32; attention AV in bf16):
  X^T via PE transposes -> QK^T = W_qkv^T X^T (transposed layout) and
  V = X W_v (natural layout); rotary applied in transposed layout via a
  pair-swap permutation matmul + elementwise combine; scores computed
  transposed ST[j,i] = K^T Q so the softmax reduction lands on the matmul
  (ones-column-augmented V yields denominators for free); out^T = [V|1]^T E;
  normalize with reciprocal; final projection W_out + bias.
"""
import sys
import numpy as np

if '/opt/trn_rl_repo' not in sys.path:
    sys.path.insert(0, '/opt/trn_rl_repo')

B, N, DIM = 8, 1024, 768
HEADS, DHEAD = 12, 64
INNER = HEADS * DHEAD           # 768
SCALE = DHEAD ** -0.5           # 0.125
NCH = N // 128                  # 8 n-chunks
KCH = DIM // 128                # 6 contraction chunks
CCH = (2 * INNER) // 128        # 12 col chunks of QK
TCH = HEADS // 2                # 6 head pairs

_CACHE = {}


def _build():
    import concourse.mybir as mybir
    from concourse import bacc
    from concourse.tile import TileContext

    F32 = mybir.dt.float32
    F32R = mybir.dt.float32r
    BF16 = mybir.dt.bfloat16
    AF = mybir.ActivationFunctionType

    nc = bacc.Bacc("TRN2", target_bir_lowering=False, debug=False, num_devices=8)

    x_d = nc.dram_tensor("x", [N, DIM], F32, kind="ExternalInput")
    pos_d = nc.dram_tensor("pos", [N, DHEAD], F32, kind="ExternalInput")
    wqkv_d = nc.dram_tensor("wqkv", [DIM, 3 * INNER], F32, kind="ExternalInput")
    wout_d = nc.dram_tensor("wout", [INNER, DIM], F32, kind="ExternalInput")
    bout_d = nc.dram_tensor("bout", [DIM], F32, kind="ExternalInput")
    y_d = nc.dram_tensor("y", [N, DIM], F32, kind="ExternalOutput")
    den_d = nc.dram_tensor("den_scr", [HEADS, N], F32)
    rcp_d = nc.dram_tensor("rcp_scr", [HEADS, N], F32)

    # ---- inline constants -------------------------------------------------
    ident_d = nc.inline_tensor(np.eye(128, dtype=np.float32), name="ident")
    # P: pair swap with negate; (P @ q) in [d, n] layout rotates pairs.
    P = np.zeros((128, 128), np.float32)
    for r in range(64):
        P[2 * r, 2 * r + 1] = -1.0
        P[2 * r + 1, 2 * r] = 1.0
    pmT_d = nc.inline_tensor(P.T.copy(), name="pmT")
    # sin128 = Rsin @ posT, cos128 = Rcos @ posT (posT rows 0:32 sin, 32:64 cos)
    RsinT = np.zeros((64, 128), np.float32)
    RcosT = np.zeros((64, 128), np.float32)
    for m in range(128):
        RsinT[(m % 64) // 2, m] = 1.0
        RcosT[32 + (m % 64) // 2, m] = 1.0
    rsinT_d = nc.inline_tensor(RsinT, name="rsinT")
    rcosT_d = nc.inline_tensor(RcosT, name="rcosT")

    with TileContext(nc) as tc:
        with tc.tile_pool(name="wp", bufs=1) as wp, \
             tc.tile_pool(name="stage", bufs=1) as stpool, \
             tc.tile_pool(name="big", bufs=1) as big, \
             tc.tile_pool(name="tp", bufs=2) as tp, \
             tc.tile_pool(name="epool", bufs=1) as epool, \
             tc.tile_pool(name="misc", bufs=1) as misc, \
             tc.tile_pool(name="psA", bufs=2, space="PSUM") as psA, \
             tc.tile_pool(name="psB", bufs=2, space="PSUM") as psB:

            # ---- input DMAs first (sync queue): x then pos
            x_sb = []
            for i in range(NCH):
                xs = tp.tile([128, DIM], F32, name=f"x_sb_{i}", tag="xy", bufs=3)
                nc.sync.dma_start(xs[:], x_d[i * 128:(i + 1) * 128, :])
                x_sb.append(xs)
            p_sb = []
            for i in range(NCH):
                ps = tp.tile([128, DHEAD], F32, name=f"p_sb_{i}", tag="pl")
                nc.sync.dma_start(ps[:], pos_d[i * 128:(i + 1) * 128, :])
                p_sb.append(ps)

            # ---- constants
            ident_sb = misc.tile([128, 128], F32, name="ident_sb", tag="ident_sb")
            nc.sync.dma_start(ident_sb[:], ident_d.ap())
            pm_f32 = misc.tile([128, 128], F32, name="pm_f32", tag="pm_f32")
            nc.sync.dma_start(pm_f32[:], pmT_d.ap())
            pm_sb = misc.tile([128, 128], F32R, name="pm_sb", tag="pm_sb")
            nc.vector.tensor_copy(pm_sb[:], pm_f32[:])
            rs_f32 = misc.tile([64, 128], F32, name="rs_f32", tag="rs_f32")
            nc.sync.dma_start(rs_f32[:], rsinT_d.ap())
            rs_sb = misc.tile([64, 128], F32R, name="rs_sb", tag="rs_sb")
            nc.vector.tensor_copy(rs_sb[:], rs_f32[:])
            rc_f32 = misc.tile([64, 128], F32, name="rc_f32", tag="rc_f32")
            nc.sync.dma_start(rc_f32[:], rcosT_d.ap())
            rc_sb = misc.tile([64, 128], F32R, name="rc_sb", tag="rc_sb")
            nc.vector.tensor_copy(rc_sb[:], rc_f32[:])
            b_row = tp.tile([1, DIM], F32, name="b_row", tag="rcp", bufs=2)
            b_bcast = misc.tile([128, DIM], F32, name="b_bcast", tag="b_bcast")
            nc.sync.dma_start(b_row[:], bout_d.ap().unsqueeze(0))
            nc.gpsimd.partition_broadcast(b_bcast[:], b_row[:])

            # ---- weights on the scalar DMA queue, staged in half-chunks
            wqkv_r = [wp.tile([128, 3 * INNER], F32R, name=f"wqkv_r_{k}",
                              tag=f"wqkv_r_{k}") for k in range(KCH)]
            wout_r = [wp.tile([128, DIM], F32R, name=f"wout_r_{k}",
                              tag=f"wout_r_{k}") for k in range(KCH)]
            for k in range(KCH):
                for hf in range(2):
                    sl = slice(hf * 1152, (hf + 1) * 1152)
                    st = stpool.tile([128, 1152], F32, name=f"wst_{k}_{hf}",
                                     tag="stage", bufs=2)
                    nc.scalar.dma_start(st[:], wqkv_d[k * 128:(k + 1) * 128, sl])
                    nc.scalar.activation(wqkv_r[k][:, sl], st[:], AF.Copy)
            for k in range(KCH):
                st = stpool.tile([128, DIM], F32, name=f"wost_{k}",
                                 tag="stage", bufs=2)
                nc.scalar.dma_start(st[:], wout_d[k * 128:(k + 1) * 128, :])
                nc.scalar.activation(wout_r[k][:], st[:], AF.Copy)

            # ---- X transposes + V per n-chunk (V uses xt cols of that chunk)
            xt = [big.tile([128, N], F32R, name=f"xt{k}", tag=f"xt{k}")
                  for k in range(KCH)]
            vaug = [big.tile([128, HEADS * 65], BF16, name=f"vaug{i}",
                             tag=f"vaug{i}") for i in range(NCH)]
            for i in range(NCH):
                nc.gpsimd.memset(vaug[i][:], 1.0)
            for i in range(NCH):
                for k in range(KCH):
                    pool = psA if (k % 2 == 0) else psB
                    pt = pool.tile([128, 128], F32, name=f"pt_{i}_{k}",
                                   tag="psA" if (k % 2 == 0) else "psB")
                    nc.tensor.transpose(pt[:], x_sb[i][:, k * 128:(k + 1) * 128],
                                        ident_sb[:])
                    nc.vector.tensor_copy(xt[k][:, i * 128:(i + 1) * 128], pt[:])

            # ---- pos -> posT -> sin128/cos128
            posT = tp.tile([64, N], F32R, name="posT", tag="rep", bufs=2)
            for i in range(NCH):
                pp = psB.tile([64, 128], F32, name=f"pp_{i}", tag="psB")
                nc.tensor.transpose(pp[:], p_sb[i][:], ident_sb[:])
                nc.vector.tensor_copy(posT[:, i * 128:(i + 1) * 128], pp[:])
            sin128 = misc.tile([128, N], F32, name="sin128", tag="sin128")
            cos128 = misc.tile([128, N], F32, name="cos128", tag="cos128")
            for half in range(2):
                sl = slice(half * 512, (half + 1) * 512)
                ps_s = psB.tile([128, 512], F32, name=f"ps_s{half}", tag="psB")
                nc.tensor.matmul(ps_s[:], rs_sb[:], posT[:, sl], start=True, stop=True)
                nc.vector.tensor_copy(sin128[:, sl], ps_s[:])
                ps_c = psB.tile([128, 512], F32, name=f"ps_c{half}", tag="psB")
                nc.tensor.matmul(ps_c[:], rc_sb[:], posT[:, sl], start=True, stop=True)
                nc.vector.tensor_copy(cos128[:, sl], ps_c[:])

            # ---- V phase (after transposes; W on the parallel queue)
            for i in range(NCH):
                acc = psA.tile([128, N], F32, name=f"vp_{i}", tag="psA")
                for half in range(2):
                    for k in range(KCH):
                        nc.tensor.matmul(
                            acc[:, half * 512:half * 512 + 384],
                            xt[k][:, i * 128:(i + 1) * 128],
                            wqkv_r[k][:, 2 * INNER + half * 384:
                                      2 * INNER + (half + 1) * 384],
                            start=(k == 0), stop=(k == KCH - 1))
                for half in range(2):
                    dst = vaug[i][:, half * 390:half * 390 + 390] \
                        .rearrange("p (h d) -> p h d", d=65)[:, :, 0:64]
                    nc.vector.tensor_copy(
                        dst, acc[:, half * 512:half * 512 + 384]
                        .rearrange("p (h d) -> p h d", d=64))

            # ---- interleaved: per pair t produce QK chunks (c=t, 6+t) with
            # rotary, then attention for pair t; the scheduler overlaps pair
            # t's attention with pair t+1's QK production.
            ao = [big.tile([128, N], F32R, name=f"ao{t}", tag=f"ao{t}")
                  for t in range(TCH)]
            qkl = {}

            def emit_qk_pair(tq):
                for c in (tq, 6 + tq):
                    qkl[(tq, c >= 6)] = qkc = big.tile([128, N], F32R, name=f"qk{c}",
                                     tag="qkA" if c < 6 else "qkB", bufs=2)
                    acc = psA.tile([128, N], F32, name=f"qkp_{c}", tag="psA")
                    for half in range(2):
                        sl = slice(half * 512, (half + 1) * 512)
                        for k in range(KCH):
                            nc.tensor.matmul(acc[:, sl],
                                             wqkv_r[k][:, c * 128:(c + 1) * 128],
                                             xt[k][:, sl],
                                             start=(k == 0), stop=(k == KCH - 1))
                    nc.vector.tensor_copy(qkc[:], acc[:])
                    # rotary in place: qk[c] <- qk[c]*cos + (P @ qk[c])*sin
                    rp = psA.tile([128, N], F32, name=f"rot_{c}", tag="psA")
                    for half in range(2):
                        sl = slice(half * 512, (half + 1) * 512)
                        nc.tensor.matmul(rp[:, sl], pm_sb[:], qkc[:, sl],
                                         start=True, stop=True)
                    nc.vector.tensor_mul(rp[:], rp[:], sin128[:])
                    nc.vector.tensor_mul(qkc[:], qkc[:], cos128[:])
                    nc.vector.tensor_add(qkc[:], qkc[:], rp[:])


            def emit_attention(t):
                # attention for pair t (one behind QK production)
                avs = [psB.tile([65, N], F32, name=f"av_{2 * t + hf}", tag="psB")
                       for hf in range(2)]
                for jc in range(NCH):
                    for half in range(2):
                        h = 2 * t + half
                        hs = slice(half * 64, (half + 1) * 64)
                        kt_slice = qkl[(t, True)][hs, jc * 128:(jc + 1) * 128]
                        e = epool.tile([128, N], BF16, name=f"e_{h}_{jc}",
                                       tag="e", bufs=4)
                        stp = psA.tile([128, N], F32, name=f"st_{h}_{jc}",
                                       tag="psA")
                        for hf in range(2):
                            sl = slice(hf * 512, (hf + 1) * 512)
                            nc.tensor.matmul(stp[:, sl], kt_slice,
                                             qkl[(t, False)][hs, sl],
                                             start=True, stop=True)
                        nc.scalar.activation(e[:], stp[:], AF.Exp, scale=SCALE)
                        v_sl = vaug[jc][:, h * 65:(h + 1) * 65]
                        for hf in range(2):
                            sl = slice(hf * 512, (hf + 1) * 512)
                            nc.tensor.matmul(avs[half][:, sl], v_sl, e[:, sl],
                                             start=(jc == 0), stop=(jc == NCH - 1))
                # normalize: den -> dram -> [128,8] recip -> dram -> row ->
                # gpsimd bcast [64,N]; multiply straight out of PSUM into ao.
                for half in range(2):
                    h = 2 * t + half
                    hs = slice(half * 64, (half + 1) * 64)
                    dr = tp.tile([1, N], F32, name=f"dr_{h}", tag="rcp", bufs=2)
                    nc.vector.tensor_copy(dr[:], avs[half][64:65, :])
                    nc.sync.dma_start(den_d.ap()[h].unsqueeze(0), dr[:])
                    dsq = tp.tile([128, 8], F32, name=f"dsq_{h}", tag="dsq",
                                  bufs=2)
                    nc.sync.dma_start(
                        dsq[:], den_d.ap()[h].rearrange("(p f) -> p f", f=8))
                    nc.vector.reciprocal(dsq[:], dsq[:])
                    nc.sync.dma_start(
                        rcp_d.ap()[h].rearrange("(p f) -> p f", f=8), dsq[:])
                    rw = tp.tile([1, N], F32, name=f"rw_{h}", tag="rcp", bufs=2)
                    nc.sync.dma_start(rw[:], rcp_d.ap()[h].unsqueeze(0))
                    rep = tp.tile([64, N], F32, name=f"rep_{h}", tag="rep",
                                  bufs=2)
                    nc.gpsimd.partition_broadcast(rep[:], rw[:], channels=64)
                    nc.vector.tensor_mul(ao[t][hs, :], avs[half][0:64, :],
                                         rep[:])


            emit_qk_pair(0)
            for t in range(1, TCH):
                emit_qk_pair(t)
                emit_attention(t - 1)
            emit_attention(TCH - 1)
            # ---- output projection + bias
            for i in range(NCH):
                op = psA.tile([128, N], F32, name=f"op_{i}", tag="psA")
                for k in range(KCH):
                    lhs = ao[k][:, i * 128:(i + 1) * 128]
                    nc.tensor.matmul(op[:, 0:512], lhs, wout_r[k][:, 0:512],
                                     start=(k == 0), stop=(k == KCH - 1))
                    nc.tensor.matmul(op[:, 512:768], lhs, wout_r[k][:, 512:768],
                                     start=(k == 0), stop=(k == KCH - 1))
                y_sb = tp.tile([128, DIM], F32, name=f"y_sb_{i}", tag="xy", bufs=3)
                nc.vector.tensor_add(y_sb[:], op[:, 0:768], b_bcast[:])
                nc.sync.dma_start(y_d[i * 128:(i + 1) * 128, :], y_sb[:])

    nc.compile()
    return nc


def get_nc():
    if 'nc' not in _CACHE:
        _CACHE['nc'] = _build()
    return _CACHE['nc']


def make_in_maps(inputs):
    x = np.ascontiguousarray(np.asarray(inputs["x"], dtype=np.float32))
    pos = np.ascontiguousarray(
        np.asarray(inputs["pos_emb"], dtype=np.float32).reshape(N, DHEAD))
    wqkv = np.ascontiguousarray(np.asarray(inputs["W_qkv"], dtype=np.float32))
    wout = np.ascontiguousarray(np.asarray(inputs["W_out"], dtype=np.float32))
    bout = np.ascontiguousarray(np.asarray(inputs["b_out"], dtype=np.float32))
    return [{"x": np.ascontiguousarray(x[i]), "pos": pos, "wqkv": wqkv,
             "wout": wout, "bout": bout} for i in range(B)]


def run(inputs, trace=False, **kwargs):
    """inputs: dict with full-shape arrays as in reference.setup_inputs()."""
    from concourse.bass_utils import run_bass_kernel_spmd
    nc = get_nc()
    res = run_bass_kernel_spmd(nc, make_in_maps(inputs),
                               core_ids=list(range(B)), trace=trace, **kwargs)
    out = np.stack([res.results[i]["y"] for i in range(B)], axis=0)
    return out, res


def kernel(**inputs):
    out, _ = run(inputs, trace=False)
    return out



# revision 2
# speedup vs baseline: 1.0766x; 1.0766x over previous
"""Rotary multi-head attention (b=8, n=1024, dim=768, heads=12, d_head=64)
on 8 Trainium2 NeuronCores, data-parallel over batch (1 batch row per core).

v2: all matmul operands bf16 (FWL fast weight loads), host-side prep:
X is pre-transposed and sin/cos rotary tables precomputed on the host and
shipped as bf16 DRAM inputs, so the device does no transposes and no
weight staging. Per-core pipeline: QK^T = W_qkv^T X^T (transposed layout),
V = X W_v (natural layout, packed into 128-col-per-head stationary tiles
with a ones column for softmax denominators); rotary via a pair-swap
permutation matmul + elementwise combine; scores transposed ST[j,i] =
K^T Q; exp on ScalarE; out^T = Vaug^T E accumulated over key chunks
(denominator rides in partition 64); normalize with reciprocal; final
projection W_out + bias in fp32.
"""
import sys
import numpy as np

if '/opt/trn_rl_repo' not in sys.path:
    sys.path.insert(0, '/opt/trn_rl_repo')

B, N, DIM = 8, 1024, 768
HEADS, DHEAD = 12, 64
INNER = HEADS * DHEAD           # 768
SCALE = DHEAD ** -0.5           # 0.125
NCH = N // 128                  # 8 n-chunks
KCH = DIM // 128                # 6 contraction chunks
TCH = HEADS // 2                # 6 head pairs

_CACHE = {}


def _build():
    import concourse.mybir as mybir
    from concourse import bacc
    from concourse.tile import TileContext

    F32 = mybir.dt.float32
    BF16 = mybir.dt.bfloat16
    AF = mybir.ActivationFunctionType

    nc = bacc.Bacc("TRN2", target_bir_lowering=False, debug=False, num_devices=8)

    xt_d = nc.dram_tensor("xt", [DIM, N], BF16, kind="ExternalInput")
    wqkv_d = nc.dram_tensor("wqkv", [DIM, 3 * INNER], BF16, kind="ExternalInput")
    wout_d = nc.dram_tensor("wout", [INNER, DIM], BF16, kind="ExternalInput")
    bout_d = nc.dram_tensor("bout", [DIM], F32, kind="ExternalInput")
    sin_d = nc.dram_tensor("sintab", [128, N], BF16, kind="ExternalInput")
    cos_d = nc.dram_tensor("costab", [128, N], BF16, kind="ExternalInput")
    y_d = nc.dram_tensor("y", [N, DIM], F32, kind="ExternalOutput")
    den_d = nc.dram_tensor("den_scr", [HEADS, N], F32)
    rcp_d = nc.dram_tensor("rcp_scr", [HEADS, N], F32)

    # P: pair swap with negate; (P @ q) in [d, n] layout rotates pairs.
    import ml_dtypes
    P = np.zeros((128, 128), np.float32)
    for r in range(64):
        P[2 * r, 2 * r + 1] = -1.0
        P[2 * r + 1, 2 * r] = 1.0
    pmT_d = nc.inline_tensor(P.T.copy().astype(ml_dtypes.bfloat16), name="pmT")

    with TileContext(nc) as tc:
        with tc.tile_pool(name="wp", bufs=1) as wp, \
             tc.tile_pool(name="big", bufs=1) as big, \
             tc.tile_pool(name="tp", bufs=2) as tp, \
             tc.tile_pool(name="epool", bufs=1) as epool, \
             tc.tile_pool(name="misc", bufs=1) as misc, \
             tc.tile_pool(name="psA", bufs=2, space="PSUM") as psA, \
             tc.tile_pool(name="psB", bufs=2, space="PSUM") as psB:

            # ---- input DMAs: sync queue carries xt + rotary tables,
            # scalar queue carries the weights; both start immediately.
            xt = [wp.tile([128, N], BF16, name=f"xt{k}", tag=f"xt{k}")
                  for k in range(KCH)]
            for k in range(KCH):
                nc.sync.dma_start(xt[k][:], xt_d[k * 128:(k + 1) * 128, :])
            sin_sb = misc.tile([128, N], BF16, name="sin_sb", tag="sin_sb")
            nc.sync.dma_start(sin_sb[:], sin_d.ap())
            cos_sb = misc.tile([128, N], BF16, name="cos_sb", tag="cos_sb")
            nc.sync.dma_start(cos_sb[:], cos_d.ap())
            pm_sb = misc.tile([128, 128], BF16, name="pm_sb", tag="pm_sb")
            nc.sync.dma_start(pm_sb[:], pmT_d.ap())

            wqkv_sb = [wp.tile([128, 3 * INNER], BF16, name=f"wqkv_{k}",
                               tag=f"wqkv_{k}") for k in range(KCH)]
            for k in range(KCH):
                nc.scalar.dma_start(wqkv_sb[k][:],
                                    wqkv_d[k * 128:(k + 1) * 128, :])
            wout_sb = [wp.tile([128, DIM], BF16, name=f"wout_{k}",
                               tag=f"wout_{k}") for k in range(KCH)]
            for k in range(KCH):
                nc.scalar.dma_start(wout_sb[k][:],
                                    wout_d[k * 128:(k + 1) * 128, :])
            b_row = tp.tile([1, DIM], F32, name="b_row", tag="rcp", bufs=2)
            b_bcast = misc.tile([128, DIM], F32, name="b_bcast", tag="b_bcast")
            nc.scalar.dma_start(b_row[:], bout_d.ap().unsqueeze(0))
            nc.gpsimd.partition_broadcast(b_bcast[:], b_row[:])

            # ---- vaug: per n-chunk, [128, h*128 + (64 V | 1 ones | 63 zero)]
            vaug = [big.tile([128, HEADS * 128], BF16, name=f"vaug{i}",
                             tag=f"vaug{i}") for i in range(NCH)]
            for i in range(NCH):
                nc.gpsimd.memset(vaug[i][:], 0.0)
                for h in range(HEADS):
                    nc.gpsimd.memset(vaug[i][:, h * 128 + 64:h * 128 + 65], 1.0)

            # ---- V phase: natural layout, stationary = xt chunk (FWL)
            for i in range(NCH):
                acc = psA.tile([128, N], F32, name=f"vp_{i}", tag="psA")
                for k in range(KCH):
                    lhs = xt[k][:, i * 128:(i + 1) * 128]
                    for half in range(2):
                        nc.tensor.matmul(
                            acc[:, half * 512:half * 512 + 384],
                            lhs,
                            wqkv_sb[k][:, 2 * INNER + half * 384:
                                       2 * INNER + (half + 1) * 384],
                            start=(k == 0), stop=(k == KCH - 1))
                for half in range(2):
                    dst = vaug[i].rearrange("p (h c) -> p h c", c=128)[
                        :, 6 * half:6 * (half + 1), 0:64]
                    nc.vector.tensor_copy(
                        dst, acc[:, half * 512:half * 512 + 384]
                        .rearrange("p (h d) -> p h d", d=64))

            # ---- interleaved: per pair t produce QK chunks (c=t, 6+t) with
            # rotary, then attention for pair t; the scheduler overlaps pair
            # t's attention with pair t+1's QK production.
            ao = [big.tile([128, N], BF16, name=f"ao{t}", tag=f"ao{t}")
                  for t in range(TCH)]
            qkl = {}

            def emit_qk_pair(tq):
                for c in (tq, 6 + tq):
                    qkc = big.tile([128, N], BF16, name=f"qk{c}",
                                   tag="qkA" if c < 6 else "qkB", bufs=2)
                    qkl[(tq, c >= 6)] = qkc
                    acc = psA.tile([128, N], F32, name=f"qkp_{c}", tag="psA")
                    for k in range(KCH):
                        lhs = wqkv_sb[k][:, c * 128:(c + 1) * 128]
                        for half in range(2):
                            sl = slice(half * 512, (half + 1) * 512)
                            nc.tensor.matmul(acc[:, sl], lhs, xt[k][:, sl],
                                             start=(k == 0), stop=(k == KCH - 1))
                    qraw = tp.tile([128, N], BF16, name=f"qraw_{c}",
                                   tag="qraw", bufs=2)
                    nc.vector.tensor_copy(qraw[:], acc[:])
                    # rotary: qkc <- qraw*cos + (P @ qraw)*sin
                    rp = psA.tile([128, N], F32, name=f"rot_{c}", tag="psA")
                    for half in range(2):
                        sl = slice(half * 512, (half + 1) * 512)
                        nc.tensor.matmul(rp[:, sl], pm_sb[:], qraw[:, sl],
                                         start=True, stop=True)
                    t1 = tp.tile([128, N], BF16, name=f"t1_{c}", tag="t1",
                                 bufs=2)
                    nc.vector.tensor_mul(t1[:], qraw[:], cos_sb[:])
                    nc.vector.tensor_mul(qkc[:], rp[:], sin_sb[:])
                    nc.vector.tensor_add(qkc[:], qkc[:], t1[:])

            def emit_attention(t):
                # attention for pair t (one behind QK production)
                qkQ = qkl[(t, False)]
                qkK = qkl[(t, True)]
                avs = [psB.tile([128, N], F32, name=f"av_{2 * t + hf}",
                                tag="psB") for hf in range(2)]
                for jc in range(NCH):
                    for half in range(2):
                        h = 2 * t + half
                        hs = slice(half * 64, (half + 1) * 64)
                        kt_slice = qkK[hs, jc * 128:(jc + 1) * 128]
                        e = epool.tile([128, N], BF16, name=f"e_{h}_{jc}",
                                       tag="e", bufs=4)
                        stp = psA.tile([128, N], F32, name=f"st_{h}_{jc}",
                                       tag="psA")
                        for hf in range(2):
                            sl = slice(hf * 512, (hf + 1) * 512)
                            nc.tensor.matmul(stp[:, sl], kt_slice,
                                             qkQ[hs, sl],
                                             start=True, stop=True)
                        nc.scalar.activation(e[:], stp[:], AF.Exp, scale=SCALE)
                        v_sl = vaug[jc][:, h * 128:(h + 1) * 128]
                        for hf in range(2):
                            sl = slice(hf * 512, (hf + 1) * 512)
                            nc.tensor.matmul(avs[half][:, sl], v_sl, e[:, sl],
                                             start=(jc == 0),
                                             stop=(jc == NCH - 1))
                # normalize: den -> dram -> [128,8] recip -> dram -> row ->
                # gpsimd bcast [64,N]; multiply straight out of PSUM into ao.
                for half in range(2):
                    h = 2 * t + half
                    hs = slice(half * 64, (half + 1) * 64)
                    dr = tp.tile([1, N], F32, name=f"dr_{h}", tag="rcp", bufs=2)
                    nc.vector.tensor_copy(dr[:], avs[half][64:65, :])
                    nc.sync.dma_start(den_d.ap()[h].unsqueeze(0), dr[:])
                    dsq = tp.tile([128, 8], F32, name=f"dsq_{h}", tag="dsq",
                                  bufs=2)
                    nc.sync.dma_start(
                        dsq[:], den_d.ap()[h].rearrange("(p f) -> p f", f=8))
                    nc.vector.reciprocal(dsq[:], dsq[:])
                    nc.sync.dma_start(
                        rcp_d.ap()[h].rearrange("(p f) -> p f", f=8), dsq[:])
                    rw = tp.tile([1, N], F32, name=f"rw_{h}", tag="rcp", bufs=2)
                    nc.sync.dma_start(rw[:], rcp_d.ap()[h].unsqueeze(0))
                    rep = tp.tile([64, N], F32, name=f"rep_{h}", tag="rep",
                                  bufs=2)
                    nc.gpsimd.partition_broadcast(rep[:], rw[:], channels=64)
                    nc.vector.tensor_mul(ao[t][hs, :], avs[half][0:64, :],
                                         rep[:])

            emit_qk_pair(0)
            for t in range(1, TCH):
                emit_qk_pair(t)
                emit_attention(t - 1)
            emit_attention(TCH - 1)

            # ---- output projection + bias (stationary = ao chunk, FWL)
            for i in range(NCH):
                op = psA.tile([128, N], F32, name=f"op_{i}", tag="psA")
                for k in range(KCH):
                    lhs = ao[k][:, i * 128:(i + 1) * 128]
                    nc.tensor.matmul(op[:, 0:512], lhs, wout_sb[k][:, 0:512],
                                     start=(k == 0), stop=(k == KCH - 1))
                    nc.tensor.matmul(op[:, 512:768], lhs, wout_sb[k][:, 512:768],
                                     start=(k == 0), stop=(k == KCH - 1))
                y_sb = tp.tile([128, DIM], F32, name=f"y_sb_{i}", tag="xy",
                               bufs=3)
                nc.vector.tensor_add(y_sb[:], op[:, 0:768], b_bcast[:])
                nc.sync.dma_start(y_d[i * 128:(i + 1) * 128, :], y_sb[:])

    nc.compile()
    return nc


def get_nc():
    if 'nc' not in _CACHE:
        _CACHE['nc'] = _build()
    return _CACHE['nc']


def make_in_maps(inputs):
    import ml_dtypes
    BF = ml_dtypes.bfloat16
    x = np.asarray(inputs["x"], dtype=np.float32)
    pos = np.asarray(inputs["pos_emb"], dtype=np.float32).reshape(N, DHEAD)
    wqkv = np.ascontiguousarray(
        np.asarray(inputs["W_qkv"], dtype=np.float32).astype(BF))
    wout = np.ascontiguousarray(
        np.asarray(inputs["W_out"], dtype=np.float32).astype(BF))
    bout = np.ascontiguousarray(np.asarray(inputs["b_out"], dtype=np.float32))
    # rotary tables in the transposed [d=128, n] layout used on-device:
    # row m of a head-half uses sin(pos[n, (m%64)//2]), cos(pos[n, 32+(m%64)//2])
    d = np.arange(128) % 64
    sintab = np.ascontiguousarray(pos[:, d // 2].T.astype(BF))
    costab = np.ascontiguousarray(pos[:, 32 + d // 2].T.astype(BF))
    return [{"xt": np.ascontiguousarray(x[i].T.astype(BF)),
             "wqkv": wqkv, "wout": wout, "bout": bout,
             "sintab": sintab, "costab": costab} for i in range(B)]


def run(inputs, trace=False, **kwargs):
    """inputs: dict with full-shape arrays as in reference.setup_inputs()."""
    from concourse.bass_utils import run_bass_kernel_spmd
    nc = get_nc()
    res = run_bass_kernel_spmd(nc, make_in_maps(inputs),
                               core_ids=list(range(B)), trace=trace, **kwargs)
    out = np.stack([res.results[i]["y"] for i in range(B)], axis=0)
    return out, res


def kernel(**inputs):
    out, _ = run(inputs, trace=False)
    return out


# revision 4
# speedup vs baseline: 1.6458x; 1.5288x over previous
"""Rotary multi-head attention (b=8, n=1024, dim=768, heads=12, d_head=64)
on 8 Trainium2 NeuronCores, data-parallel over batch (1 batch row per core).

v3: fp16 operands on the scores path (10-bit mantissa ~ tf32 accuracy, but
2-byte so every 128-col stationary gets the fast-weight-load path), bf16 for
exp outputs / V (exp can overflow fp16 range). Host-side prep: X shipped
pre-transposed, rotary sin/cos tables precomputed, weights pre-cast — the
device does no transposes and no weight staging. Attention runs per head
with a 3-tag PSUM budget (scores x2 | AV accumulator | matmul accumulator)
and the AV accumulator is freed early via a PSUM->SBUF copy so heads
pipeline; QK production for pair t+1 fills PE gaps under pair t's softmax.
"""
import sys
import numpy as np

if '/opt/trn_rl_repo' not in sys.path:
    sys.path.insert(0, '/opt/trn_rl_repo')

B, N, DIM = 8, 1024, 768
HEADS, DHEAD = 12, 64
INNER = HEADS * DHEAD           # 768
SCALE = DHEAD ** -0.5           # 0.125
NCH = N // 128                  # 8 n-chunks
KCH = DIM // 128                # 6 contraction chunks
TCH = HEADS // 2                # 6 head pairs

_CACHE = {}


def _build():
    import concourse.mybir as mybir
    from concourse import bacc
    from concourse.tile import TileContext

    F32 = mybir.dt.float32
    F16 = mybir.dt.float16
    BF16 = mybir.dt.bfloat16
    AF = mybir.ActivationFunctionType

    nc = bacc.Bacc("TRN2", target_bir_lowering=False, debug=False, num_devices=8)

    xt_d = nc.dram_tensor("xt", [DIM, N], F16, kind="ExternalInput")
    wqkv_d = nc.dram_tensor("wqkv", [DIM, 3 * INNER], F16, kind="ExternalInput")
    wout_d = nc.dram_tensor("wout", [INNER, DIM], F16, kind="ExternalInput")
    bout_d = nc.dram_tensor("bout", [DIM], F32, kind="ExternalInput")
    sin_d = nc.dram_tensor("sintab", [128, N], F16, kind="ExternalInput")
    cos_d = nc.dram_tensor("costab", [128, N], F16, kind="ExternalInput")
    y_d = nc.dram_tensor("y", [N, DIM], F32, kind="ExternalOutput")
    den_d = nc.dram_tensor("den_scr", [HEADS, N], F32)
    rcp_d = nc.dram_tensor("rcp_scr", [HEADS, N], F32)

    # P: pair swap with negate; (P @ q) in [d, n] layout rotates pairs.
    P = np.zeros((128, 128), np.float32)
    for r in range(64):
        P[2 * r, 2 * r + 1] = -1.0
        P[2 * r + 1, 2 * r] = 1.0
    pmT_d = nc.inline_tensor(P.T.copy().astype(np.float16), name="pmT")

    with TileContext(nc) as tc:
        with tc.tile_pool(name="wp", bufs=1) as wp, \
             tc.tile_pool(name="big", bufs=1) as big, \
             tc.tile_pool(name="tp", bufs=2) as tp, \
             tc.tile_pool(name="epool", bufs=1) as epool, \
             tc.tile_pool(name="misc", bufs=1) as misc, \
             tc.tile_pool(name="ps", bufs=1, space="PSUM") as ps:

            def psum(name, tag, bufs):
                return ps.tile([128, N], F32, name=name, tag=tag, bufs=bufs)

            # ---- input DMAs: sync queue carries xt + rotary tables,
            # scalar queue carries the weights; both start immediately.
            xt = [wp.tile([128, N], F16, name=f"xt{k}", tag=f"xt{k}")
                  for k in range(KCH)]
            for k in range(KCH):
                nc.sync.dma_start(xt[k][:], xt_d[k * 128:(k + 1) * 128, :])
            sin_sb = misc.tile([128, N], F16, name="sin_sb", tag="sin_sb")
            nc.sync.dma_start(sin_sb[:], sin_d.ap())
            cos_sb = misc.tile([128, N], F16, name="cos_sb", tag="cos_sb")
            nc.sync.dma_start(cos_sb[:], cos_d.ap())
            pm_sb = misc.tile([128, 128], F16, name="pm_sb", tag="pm_sb")
            nc.sync.dma_start(pm_sb[:], pmT_d.ap())

            wqkv_sb = [wp.tile([128, 3 * INNER], F16, name=f"wqkv_{k}",
                               tag=f"wqkv_{k}") for k in range(KCH)]
            for k in range(KCH):
                nc.scalar.dma_start(wqkv_sb[k][:],
                                    wqkv_d[k * 128:(k + 1) * 128, :])
            wout_sb = [wp.tile([128, DIM], F16, name=f"wout_{k}",
                               tag=f"wout_{k}") for k in range(KCH)]
            for k in range(KCH):
                nc.scalar.dma_start(wout_sb[k][:],
                                    wout_d[k * 128:(k + 1) * 128, :])
            b_row = tp.tile([1, DIM], F32, name="b_row", tag="rcp", bufs=2)
            b_bcast = misc.tile([128, DIM], F32, name="b_bcast", tag="b_bcast")
            nc.scalar.dma_start(b_row[:], bout_d.ap().unsqueeze(0))
            nc.gpsimd.partition_broadcast(b_bcast[:], b_row[:])

            # ---- vaug: per n-chunk, [128, h*128 + (64 V | 1 ones | 63 zero)]
            vaug = [big.tile([128, HEADS * 128], BF16, name=f"vaug{i}",
                             tag=f"vaug{i}") for i in range(NCH)]
            for i in range(NCH):
                nc.gpsimd.memset(vaug[i][:], 0.0)
                for h in range(HEADS):
                    nc.gpsimd.memset(vaug[i][:, h * 128 + 64:h * 128 + 65], 1.0)

            # ---- V phase: natural layout, stationary = xt chunk (FWL);
            # ping-pong between the acc and (not-yet-used) avs PSUM tags so
            # chunk i+1's matmuls overlap chunk i's PSUM->SBUF copies.
            for i in range(NCH):
                acc = psum(f"vp_{i}", "acc" if i % 2 == 0 else "avs", 1)
                for k in range(KCH):
                    lhs = xt[k][:, i * 128:(i + 1) * 128]
                    for half in range(2):
                        nc.tensor.matmul(
                            acc[:, half * 512:half * 512 + 384],
                            lhs,
                            wqkv_sb[k][:, 2 * INNER + half * 384:
                                       2 * INNER + (half + 1) * 384],
                            start=(k == 0), stop=(k == KCH - 1))
                for half in range(2):
                    dst = vaug[i].rearrange("p (h c) -> p h c", c=128)[
                        :, 6 * half:6 * (half + 1), 0:64]
                    nc.vector.tensor_copy(
                        dst, acc[:, half * 512:half * 512 + 384]
                        .rearrange("p (h d) -> p h d", d=64))

            # ---- interleaved: per pair t produce QK chunks (c=t, 6+t) with
            # rotary, then attention for pair t; the scheduler overlaps pair
            # t's attention with pair t+1's QK production.
            ao = [big.tile([128, N], F16, name=f"ao{t}", tag=f"ao{t}")
                  for t in range(TCH)]
            qkl = {}

            def emit_qk_pair(tq):
                for c in (tq, 6 + tq):
                    qkc = big.tile([128, N], F16, name=f"qk{c}",
                                   tag="qkA" if c < 6 else "qkB", bufs=2)
                    qkl[(tq, c >= 6)] = qkc
                    acc = psum(f"qkp_{c}", "acc", 1)
                    for k in range(KCH):
                        lhs = wqkv_sb[k][:, c * 128:(c + 1) * 128]
                        for half in range(2):
                            sl = slice(half * 512, (half + 1) * 512)
                            nc.tensor.matmul(acc[:, sl], lhs, xt[k][:, sl],
                                             start=(k == 0), stop=(k == KCH - 1))
                    qraw = tp.tile([128, N], F16, name=f"qraw_{c}",
                                   tag="qraw", bufs=2)
                    nc.vector.tensor_copy(qraw[:], acc[:])
                    # rotary: qkc <- qraw*cos + (P @ qraw)*sin
                    rp = psum(f"rot_{c}", "acc", 1)
                    for half in range(2):
                        sl = slice(half * 512, (half + 1) * 512)
                        nc.tensor.matmul(rp[:, sl], pm_sb[:], qraw[:, sl],
                                         start=True, stop=True)
                    t1 = tp.tile([128, N], F16, name=f"t1_{c}", tag="t1",
                                 bufs=2)
                    nc.vector.tensor_mul(t1[:], qraw[:], cos_sb[:])
                    nc.vector.tensor_mul(qkc[:], rp[:], sin_sb[:])
                    nc.vector.tensor_add(qkc[:], qkc[:], t1[:])

            def emit_attention(t):
                # attention for pair t, one head at a time (one behind QK
                # production). avs is freed early via the av_sb copy so the
                # next head's accumulation overlaps this head's normalize.
                qkQ = qkl[(t, False)]
                qkK = qkl[(t, True)]
                for half in range(2):
                    h = 2 * t + half
                    hs = slice(half * 64, (half + 1) * 64)
                    avs = psum(f"av_{h}", "avs", 1)
                    for jc in range(NCH):
                        kt_slice = qkK[hs, jc * 128:(jc + 1) * 128]
                        e = epool.tile([128, N], BF16, name=f"e_{h}_{jc}",
                                       tag="e", bufs=4)
                        stp = psum(f"st_{h}_{jc}", "stp", 2)
                        for hf in range(2):
                            sl = slice(hf * 512, (hf + 1) * 512)
                            nc.tensor.matmul(stp[:, sl], kt_slice,
                                             qkQ[hs, sl],
                                             start=True, stop=True)
                        nc.scalar.activation(e[:], stp[:], AF.Exp, scale=SCALE)
                        v_sl = vaug[jc][:, h * 128:(h + 1) * 128]
                        for hf in range(2):
                            sl = slice(hf * 512, (hf + 1) * 512)
                            nc.tensor.matmul(avs[:, sl], v_sl, e[:, sl],
                                             start=(jc == 0),
                                             stop=(jc == NCH - 1))
                    # early evacuation: rows 0:64 = out^T, row 64 = denom
                    av_sb = tp.tile([65, N], F32, name=f"avsb_{h}",
                                    tag="avsb", bufs=2)
                    nc.vector.tensor_copy(av_sb[:], avs[0:65, :])
                    # normalize: den -> dram -> [128,8] recip -> dram -> row
                    # -> gpsimd bcast [64,N]; multiply into ao (fp16).
                    nc.sync.dma_start(den_d.ap()[h].unsqueeze(0),
                                      av_sb[64:65, :])
                    dsq = tp.tile([128, 8], F32, name=f"dsq_{h}", tag="dsq",
                                  bufs=2)
                    nc.sync.dma_start(
                        dsq[:], den_d.ap()[h].rearrange("(p f) -> p f", f=8))
                    nc.vector.reciprocal(dsq[:], dsq[:])
                    nc.sync.dma_start(
                        rcp_d.ap()[h].rearrange("(p f) -> p f", f=8), dsq[:])
                    rw = tp.tile([1, N], F32, name=f"rw_{h}", tag="rcp",
                                 bufs=2)
                    nc.sync.dma_start(rw[:], rcp_d.ap()[h].unsqueeze(0))
                    rep = tp.tile([64, N], F32, name=f"rep_{h}", tag="rep",
                                  bufs=2)
                    nc.gpsimd.partition_broadcast(rep[:], rw[:], channels=64)
                    nc.vector.tensor_mul(ao[t][hs, :], av_sb[0:64, :], rep[:])

            emit_qk_pair(0)
            for t in range(1, TCH):
                emit_qk_pair(t)
                emit_attention(t - 1)
            emit_attention(TCH - 1)

            # ---- output projection + bias (stationary = ao chunk, FWL);
            # ping-pong acc/stp tags (attention is done, stp is free).
            for i in range(NCH):
                op = psum(f"op_{i}", "acc" if i % 2 == 0 else "stp",
                          1 if i % 2 == 0 else 2)
                for k in range(KCH):
                    lhs = ao[k][:, i * 128:(i + 1) * 128]
                    nc.tensor.matmul(op[:, 0:512], lhs, wout_sb[k][:, 0:512],
                                     start=(k == 0), stop=(k == KCH - 1))
                    nc.tensor.matmul(op[:, 512:768], lhs, wout_sb[k][:, 512:768],
                                     start=(k == 0), stop=(k == KCH - 1))
                y_sb = tp.tile([128, DIM], F32, name=f"y_sb_{i}", tag="xy",
                               bufs=3)
                nc.vector.tensor_add(y_sb[:], op[:, 0:768], b_bcast[:])
                nc.sync.dma_start(y_d[i * 128:(i + 1) * 128, :], y_sb[:])

    nc.compile()
    return nc


def get_nc():
    if 'nc' not in _CACHE:
        _CACHE['nc'] = _build()
    return _CACHE['nc']


def make_in_maps(inputs):
    F16 = np.float16
    x = np.asarray(inputs["x"], dtype=np.float32)
    pos = np.asarray(inputs["pos_emb"], dtype=np.float32).reshape(N, DHEAD)
    wqkv = np.ascontiguousarray(
        np.asarray(inputs["W_qkv"], dtype=np.float32).astype(F16))
    wout = np.ascontiguousarray(
        np.asarray(inputs["W_out"], dtype=np.float32).astype(F16))
    bout = np.ascontiguousarray(np.asarray(inputs["b_out"], dtype=np.float32))
    # rotary tables in the transposed [d=128, n] layout used on-device:
    # row m of a head-half uses sin(pos[n, (m%64)//2]), cos(pos[n, 32+(m%64)//2])
    d = np.arange(128) % 64
    sintab = np.ascontiguousarray(pos[:, d // 2].T.astype(F16))
    costab = np.ascontiguousarray(pos[:, 32 + d // 2].T.astype(F16))
    return [{"xt": np.ascontiguousarray(x[i].T.astype(F16)),
             "wqkv": wqkv, "wout": wout, "bout": bout,
             "sintab": sintab, "costab": costab} for i in range(B)]


def run(inputs, trace=False, **kwargs):
    """inputs: dict with full-shape arrays as in reference.setup_inputs()."""
    from concourse.bass_utils import run_bass_kernel_spmd
    nc = get_nc()
    res = run_bass_kernel_spmd(nc, make_in_maps(inputs),
                               core_ids=list(range(B)), trace=trace, **kwargs)
    out = np.stack([res.results[i]["y"] for i in range(B)], axis=0)
    return out, res


def kernel(**inputs):
    out, _ = run(inputs, trace=False)
    return out


# revision 8
# speedup vs baseline: 1.6851x; 1.0239x over previous
"""Rotary multi-head attention (b=8, n=1024, dim=768, heads=12, d_head=64)
on 8 Trainium2 NeuronCores, data-parallel over batch (1 batch row per core).

v3: fp16 operands on the scores path (10-bit mantissa ~ tf32 accuracy, but
2-byte so every 128-col stationary gets the fast-weight-load path), bf16 for
exp outputs / V (exp can overflow fp16 range). Host-side prep: X shipped
pre-transposed, rotary sin/cos tables precomputed, weights pre-cast — the
device does no transposes and no weight staging. Attention runs per head
with a 3-tag PSUM budget (scores x2 | AV accumulator | matmul accumulator)
and the AV accumulator is freed early via a PSUM->SBUF copy so heads
pipeline; QK production for pair t+1 fills PE gaps under pair t's softmax.
"""
import sys
import numpy as np

if '/opt/trn_rl_repo' not in sys.path:
    sys.path.insert(0, '/opt/trn_rl_repo')

B, N, DIM = 8, 1024, 768
HEADS, DHEAD = 12, 64
INNER = HEADS * DHEAD           # 768
SCALE = DHEAD ** -0.5           # 0.125
NCH = N // 128                  # 8 n-chunks
KCH = DIM // 128                # 6 contraction chunks
TCH = HEADS // 2                # 6 head pairs

_CACHE = {}


def _build():
    import concourse.mybir as mybir
    from concourse import bacc
    from concourse.tile import TileContext

    F32 = mybir.dt.float32
    F16 = mybir.dt.float16
    BF16 = mybir.dt.bfloat16
    AF = mybir.ActivationFunctionType

    nc = bacc.Bacc("TRN2", target_bir_lowering=False, debug=False, num_devices=8)

    xt_d = nc.dram_tensor("xt", [DIM, N], F16, kind="ExternalInput")
    wqkv_d = nc.dram_tensor("wqkv", [DIM, 3 * INNER], F16, kind="ExternalInput")
    wout_d = nc.dram_tensor("wout", [INNER, DIM], F16, kind="ExternalInput")
    bout_d = nc.dram_tensor("bout", [DIM], F32, kind="ExternalInput")
    sin_d = nc.dram_tensor("sintab", [128, N], F16, kind="ExternalInput")
    cos_d = nc.dram_tensor("costab", [128, N], F16, kind="ExternalInput")
    y_d = nc.dram_tensor("y", [N, DIM], F32, kind="ExternalOutput")
    den_d = nc.dram_tensor("den_scr", [HEADS, N], F32)
    rcp_d = nc.dram_tensor("rcp_scr", [HEADS, N], F32)

    with TileContext(nc) as tc:
        with tc.tile_pool(name="wp", bufs=1) as wp, \
             tc.tile_pool(name="big", bufs=1) as big, \
             tc.tile_pool(name="tp", bufs=2) as tp, \
             tc.tile_pool(name="epool", bufs=1) as epool, \
             tc.tile_pool(name="misc", bufs=1) as misc, \
             tc.tile_pool(name="ps", bufs=1, space="PSUM") as ps:

            def psum(name, tag, bufs):
                return ps.tile([128, N], F32, name=name, tag=tag, bufs=bufs)

            # ---- input DMAs, split across both hwdge queues for fast start
            q_of = [nc.sync, nc.scalar]
            xt = [wp.tile([128, N], F16, name=f"xt{k}", tag=f"xt{k}")
                  for k in range(KCH)]
            for k in range(KCH):
                q_of[k % 2].dma_start(xt[k][:], xt_d[k * 128:(k + 1) * 128, :])
            sin_sb = misc.tile([128, N], F16, name="sin_sb", tag="sin_sb")
            nc.sync.dma_start(sin_sb[:], sin_d.ap())
            cos_sb = misc.tile([128, N], F16, name="cos_sb", tag="cos_sb")
            nc.scalar.dma_start(cos_sb[:], cos_d.ap())

            wqkv_sb = [wp.tile([128, 3 * INNER], F16, name=f"wqkv_{k}",
                               tag=f"wqkv_{k}") for k in range(KCH)]
            for k in range(KCH):
                q_of[k % 2].dma_start(wqkv_sb[k][:],
                                      wqkv_d[k * 128:(k + 1) * 128, :])
            wout_sb = [wp.tile([128, DIM], F16, name=f"wout_{k}",
                               tag=f"wout_{k}") for k in range(KCH)]
            for k in range(KCH):
                q_of[k % 2].dma_start(wout_sb[k][:],
                                      wout_d[k * 128:(k + 1) * 128, :])
            b_row = tp.tile([1, DIM], F32, name="b_row", tag="rcp", bufs=2)
            b_bcast = misc.tile([128, DIM], F32, name="b_bcast", tag="b_bcast")
            nc.scalar.dma_start(b_row[:], bout_d.ap().unsqueeze(0))
            nc.gpsimd.partition_broadcast(b_bcast[:], b_row[:])

            # ---- vaug: per n-chunk, [128, h*128 + (64 V | 1 ones | 63 zero)]
            vaug = [big.tile([128, HEADS * 128], BF16, name=f"vaug{i}",
                             tag=f"vaug{i}") for i in range(NCH)]
            for i in range(NCH):
                nc.gpsimd.memset(vaug[i][:], 0.0)
                for h in range(HEADS):
                    nc.gpsimd.memset(vaug[i][:, h * 128 + 64:h * 128 + 65], 1.0)

            # ---- V phase: natural layout, stationary = xt chunk (FWL);
            # ping-pong between the acc and (not-yet-used) avs PSUM tags so
            # chunk i+1's matmuls overlap chunk i's PSUM->SBUF copies.
            for i in range(NCH):
                acc = psum(f"vp_{i}", "acc" if i % 2 == 0 else "avs", 1)
                for k in range(KCH):
                    lhs = xt[k][:, i * 128:(i + 1) * 128]
                    for half in range(2):
                        nc.tensor.matmul(
                            acc[:, half * 512:half * 512 + 384],
                            lhs,
                            wqkv_sb[k][:, 2 * INNER + half * 384:
                                       2 * INNER + (half + 1) * 384],
                            start=(k == 0), stop=(k == KCH - 1))
                for half in range(2):
                    dst = vaug[i].rearrange("p (h c) -> p h c", c=128)[
                        :, 6 * half:6 * (half + 1), 0:64]
                    nc.vector.tensor_copy(
                        dst, acc[:, half * 512:half * 512 + 384]
                        .rearrange("p (h d) -> p h d", d=64))

            # ---- interleaved: per pair t produce QK chunks (c=t, 6+t) with
            # rotary, then attention for pair t; the scheduler overlaps pair
            # t's attention with pair t+1's QK production.
            ao = [big.tile([128, N], F16, name=f"ao{t}", tag=f"ao{t}")
                  for t in range(TCH)]
            qkl = {}

            def emit_qk_pair(tq):
                for c in (tq, 6 + tq):
                    qkc = big.tile([128, N], F16, name=f"qk{c}",
                                   tag="qkA" if c < 6 else "qkB", bufs=2)
                    qkl[(tq, c >= 6)] = qkc
                    acc = psum(f"qkp_{c}", "acc", 1)
                    for k in range(KCH):
                        lhs = wqkv_sb[k][:, c * 128:(c + 1) * 128]
                        for half in range(2):
                            sl = slice(half * 512, (half + 1) * 512)
                            nc.tensor.matmul(acc[:, sl], lhs, xt[k][:, sl],
                                             start=(k == 0), stop=(k == KCH - 1))
                    qraw = tp.tile([128, N], F16, name=f"qraw_{c}",
                                   tag="qraw", bufs=2)
                    nc.vector.tensor_copy(qraw[:], acc[:])
                    # rotary: qkc <- qraw*cos + swap(qraw)*sin_signed, where
                    # the pair swap is two partition-strided SBUF DMAs and
                    # the per-row sign of rotate_every_two is folded into
                    # the host-built sin table.
                    rps = tp.tile([128, N], F16, name=f"rps_{c}", tag="rps",
                                  bufs=2)
                    qv = qraw.rearrange("(p s) n -> p s n", s=2)
                    rv = rps.rearrange("(p s) n -> p s n", s=2)
                    nc.scalar.dma_start(rv[:, 0, :], qv[:, 1, :])
                    nc.scalar.dma_start(rv[:, 1, :], qv[:, 0, :])
                    t1 = tp.tile([128, N], F16, name=f"t1_{c}", tag="t1",
                                 bufs=2)
                    nc.vector.tensor_mul(t1[:], qraw[:], cos_sb[:])
                    nc.vector.tensor_mul(qkc[:], rps[:], sin_sb[:])
                    nc.vector.tensor_add(qkc[:], qkc[:], t1[:])

            def emit_attention(t):
                # attention for pair t, one head at a time (one behind QK
                # production). avs is freed early via the av_sb copy so the
                # next head's accumulation overlaps this head's normalize.
                qkQ = qkl[(t, False)]
                qkK = qkl[(t, True)]
                for half in range(2):
                    h = 2 * t + half
                    hs = slice(half * 64, (half + 1) * 64)
                    avs = psum(f"av_{h}", "avs", 1)
                    for jc in range(NCH):
                        kt_slice = qkK[hs, jc * 128:(jc + 1) * 128]
                        e = epool.tile([128, N], BF16, name=f"e_{h}_{jc}",
                                       tag="e", bufs=4)
                        stp = psum(f"st_{h}_{jc}", "stp", 2)
                        for hf in range(2):
                            sl = slice(hf * 512, (hf + 1) * 512)
                            nc.tensor.matmul(stp[:, sl], kt_slice,
                                             qkQ[hs, sl],
                                             start=True, stop=True)
                        nc.scalar.activation(e[:], stp[:], AF.Exp, scale=SCALE)
                        v_sl = vaug[jc][:, h * 128:(h + 1) * 128]
                        for hf in range(2):
                            sl = slice(hf * 512, (hf + 1) * 512)
                            nc.tensor.matmul(avs[:, sl], v_sl, e[:, sl],
                                             start=(jc == 0),
                                             stop=(jc == NCH - 1))
                    # early evacuation: rows 0:64 = out^T, row 64 = denom
                    av_sb = tp.tile([65, N], F32, name=f"avsb_{h}",
                                    tag="avsb", bufs=2)
                    nc.vector.tensor_copy(av_sb[:], avs[0:65, :])
                    # normalize: den -> dram -> [128,8] recip -> dram -> row
                    # -> gpsimd bcast [64,N]; multiply into ao (fp16).
                    nc.sync.dma_start(den_d.ap()[h].unsqueeze(0),
                                      av_sb[64:65, :])
                    dsq = tp.tile([128, 8], F32, name=f"dsq_{h}", tag="dsq",
                                  bufs=2)
                    nc.sync.dma_start(
                        dsq[:], den_d.ap()[h].rearrange("(p f) -> p f", f=8))
                    nc.vector.reciprocal(dsq[:], dsq[:])
                    nc.sync.dma_start(
                        rcp_d.ap()[h].rearrange("(p f) -> p f", f=8), dsq[:])
                    rw = tp.tile([1, N], F32, name=f"rw_{h}", tag="rcp",
                                 bufs=2)
                    nc.sync.dma_start(rw[:], rcp_d.ap()[h].unsqueeze(0))
                    rep = tp.tile([64, N], F32, name=f"rep_{h}", tag="rep",
                                  bufs=2)
                    nc.gpsimd.partition_broadcast(rep[:], rw[:], channels=64)
                    nc.vector.tensor_mul(ao[t][hs, :], av_sb[0:64, :], rep[:])

            emit_qk_pair(0)
            for t in range(1, TCH):
                emit_qk_pair(t)
                emit_attention(t - 1)
            emit_attention(TCH - 1)

            # ---- output projection + bias (stationary = ao chunk, FWL);
            # ping-pong acc/stp tags (attention is done, stp is free).
            for i in range(NCH):
                op = psum(f"op_{i}", "acc" if i % 2 == 0 else "stp",
                          1 if i % 2 == 0 else 2)
                for k in range(KCH):
                    lhs = ao[k][:, i * 128:(i + 1) * 128]
                    nc.tensor.matmul(op[:, 0:512], lhs, wout_sb[k][:, 0:512],
                                     start=(k == 0), stop=(k == KCH - 1))
                    nc.tensor.matmul(op[:, 512:768], lhs, wout_sb[k][:, 512:768],
                                     start=(k == 0), stop=(k == KCH - 1))
                y_sb = tp.tile([128, DIM], F32, name=f"y_sb_{i}", tag="xy",
                               bufs=3)
                nc.vector.tensor_add(y_sb[:], op[:, 0:768], b_bcast[:])
                nc.sync.dma_start(y_d[i * 128:(i + 1) * 128, :], y_sb[:])

    nc.compile()
    return nc


def get_nc():
    if 'nc' not in _CACHE:
        _CACHE['nc'] = _build()
    return _CACHE['nc']


def make_in_maps(inputs):
    F16 = np.float16
    x = np.asarray(inputs["x"], dtype=np.float32)
    pos = np.asarray(inputs["pos_emb"], dtype=np.float32).reshape(N, DHEAD)
    wqkv = np.ascontiguousarray(
        np.asarray(inputs["W_qkv"], dtype=np.float32).astype(F16))
    wout = np.ascontiguousarray(
        np.asarray(inputs["W_out"], dtype=np.float32).astype(F16))
    bout = np.ascontiguousarray(np.asarray(inputs["b_out"], dtype=np.float32))
    # rotary tables in the transposed [d=128, n] layout used on-device:
    # row m of a head-half uses sin(pos[n, (m%64)//2]), cos(pos[n, 32+(m%64)//2]).
    # rotate_every_two's sign pattern (-odd, +even source) is folded into the
    # sin table since the device does an unsigned pair-swap copy.
    d = np.arange(128) % 64
    sgn = np.where(np.arange(128) % 2 == 0, -1.0, 1.0).astype(np.float32)
    sintab = np.ascontiguousarray((sgn[:, None] * pos[:, d // 2].T).astype(F16))
    costab = np.ascontiguousarray(pos[:, 32 + d // 2].T.astype(F16))
    return [{"xt": np.ascontiguousarray(x[i].T.astype(F16)),
             "wqkv": wqkv, "wout": wout, "bout": bout,
             "sintab": sintab, "costab": costab} for i in range(B)]


def run(inputs, trace=False, **kwargs):
    """inputs: dict with full-shape arrays as in reference.setup_inputs()."""
    from concourse.bass_utils import run_bass_kernel_spmd
    nc = get_nc()
    res = run_bass_kernel_spmd(nc, make_in_maps(inputs),
                               core_ids=list(range(B)), trace=trace, **kwargs)
    out = np.stack([res.results[i]["y"] for i in range(B)], axis=0)
    return out, res


def kernel(**inputs):
    out, _ = run(inputs, trace=False)
    return out
